# revision 1
# baseline (speedup 1.0000x reference)
"""MoE positionwise FFN (top-2 of 8 experts) on 8 TRN2 NeuronCores.

Strategy: expert-parallel. Each core owns one expert e:
  - computes the router for ALL S=4096 tokens on-device (bf16 hi/lo
    3-term split => fp32-accurate logits; gate weights permuted per core
    so that col 0 of the logits is the core's own expert),
  - compacts the ids of tokens routed to its expert (sparse_gather),
  - indirect-gathers those token rows (dma_gather, bf16, 3 pieces),
  - runs the expert FFN (D->F relu F->D) in bf16 on the gathered tokens,
  - applies bias + gate weight on-device, writes compact output + ids.
Host merges: scatter-add each core's compact output into the full (B,T,D).

v2: routing matmuls are gw-stationary bf16 N=512 (was fp32 N=8 x-stationary,
which ran LDWEIGHTS-bound at cold clock for 110us); TensorE is pre-warmed;
DMA ordering x -> w1 -> w2 on the sync queue; compaction hops on the scalar
queue; token-gather runs before the gate-gather chain on GPSIMD.

Self-contained: hardcodes shapes for B=2,T=2048,D=1024,F=4096,E=8,K=2.
"""
import math

import numpy as np
import ml_dtypes

S = 4096
D = 1024
F = 4096
E = 8
NTD = D // 128   # 8 d-tiles
NTF = F // 128   # 32 f-tiles
MT = S // 128    # 32 token tiles
SENT = S         # sentinel token id (zero row)
XROWS = S + 128  # padded x rows
CSW = S + 4      # combine source width (sentinel slot S)
RCH = 512        # routing token chunk (MM group)
XCH = 512        # x DMA chunk (tokens per dma_start)
TB = 192         # FFN token block
GP = 384         # gather piece (num_idxs per dma_gather, %128==0)

_cache: dict = {}
LAST_RES = None


def _build(C: int, stage: str = "full"):
    import concourse.bacc as bacc
    import concourse.tile as tile
    import concourse.mybir as mybir
    import concourse.bass as bass
    from concourse.tile import add_dep_helper

    f32 = mybir.dt.float32
    bf16 = mybir.dt.bfloat16
    i16 = mybir.dt.int16
    AX = mybir.AxisListType
    OP = mybir.AluOpType
    ACT = mybir.ActivationFunctionType

    nc = bacc.Bacc("TRN2", target_bir_lowering=False, debug=False, num_devices=8)

    xhT_d = nc.dram_tensor("xhT", [D, S], bf16, kind="ExternalInput")
    xlT_d = nc.dram_tensor("xlT", [D, S], bf16, kind="ExternalInput")
    xb_d = nc.dram_tensor("xb", [XROWS, D], bf16, kind="ExternalInput")
    ghl_d = nc.dram_tensor("ghl", [D, 2 * E], bf16, kind="ExternalInput")
    idn_d = nc.dram_tensor("idn", [16, E], f32, kind="ExternalInput")
    w1_d = nc.dram_tensor("w1t", [D, F], bf16, kind="ExternalInput")
    w2_d = nc.dram_tensor("w2t", [F, D], bf16, kind="ExternalInput")
    b1_d = nc.dram_tensor("b1", [F], f32, kind="ExternalInput")
    b2_d = nc.dram_tensor("b2", [D], f32, kind="ExternalInput")

    yg_d = nc.dram_tensor("yg", [128, NTD, C], bf16, kind="ExternalOutput")
    idx_d = nc.dram_tensor("idx", [16, C // 16], i16, kind="ExternalOutput")

    NBLK = C // TB
    NGP = C // GP
    assert C % TB == 0 and C % GP == 0

    with tile.TileContext(nc) as tc:
        with (
            tc.tile_pool(name="wpool", bufs=1) as wpool,
            tc.tile_pool(name="xr", bufs=2) as xr,
            tc.tile_pool(name="route", bufs=1) as route,
            tc.tile_pool(name="small", bufs=1) as small,
            tc.tile_pool(name="ypool", bufs=2) as ypool,
            tc.tile_pool(name="psH", bufs=2, space="PSUM") as psH,
            tc.tile_pool(name="psY", bufs=2, space="PSUM") as psY,
            tc.tile_pool(name="psL", bufs=1, space="PSUM") as psL,
            tc.tile_pool(name="psT", bufs=1, space="PSUM") as psT,
            tc.tile_pool(name="psW", bufs=1, space="PSUM") as psW,
            tc.tile_pool(name="dram", bufs=1, space="DRAM") as dram,
        ):
            # ---------------- TensorE pre-warm (HAM) + EXP table preload ----
            junk = small.tile([128, 256], bf16)
            nc.vector.memset(junk[:], 0.0)
            # full-width (M=128) matmuls: thin outputs do not trip the HAM
            # activity monitor and the PE stays cold-clocked.
            wps = psW.tile([128, 256], f32)
            for _ in range(16):
                nc.tensor.matmul(wps[:], lhsT=junk[:, 0:128], rhs=junk[:],
                                 start=True, stop=True)
            # dummy invocations of the GPSIMD ext-isa kernels used on the
            # compaction critical path: each pays a ~6us invisible IRAM
            # library load on first use. sparse_gather and dma_gather live in
            # different libraries that evict each other — run the gather
            # dummy FIRST and the sparse_gather dummy LAST so the
            # sparse_gather library is resident when the real sg runs.
            jgidx = small.tile([128, 8], i16)
            nc.vector.memset(jgidx[:], 0.0)
            # tiny payload (256B/row): warms the library without flooding the
            # SDMA engines with descriptors while the x stream runs
            jxg = small.tile([128, 1, 128], bf16)
            nc.gpsimd.dma_gather(
                out_ap=jxg[:], in_ap=xb_d[:, 0:128], idxs_ap=jgidx[:],
                num_idxs=128, num_idxs_reg=128, elem_size=128, elem_step=D,
                transpose=True, single_packet=False,
            )
            jidx = small.tile([16, 16], i16)
            nc.vector.memset(jidx[:], -1.0)
            jsg = small.tile([16, 16], i16)
            jnf = small.tile([1, 1], mybir.dt.uint32)
            jsg1 = nc.gpsimd.sparse_gather(jsg[:], jidx[:], num_found=jnf[:])

            # ---------------- small loads (scalar queue, front) -------------
            ghl_sb = small.tile([128, NTD, 2 * E], bf16)
            nc.scalar.dma_start(out=ghl_sb[:], in_=ghl_d[:, :].rearrange("(dt p) e -> p dt e", p=128))
            idn_sb = small.tile([16, E], f32)
            nc.scalar.dma_start(out=idn_sb[:], in_=idn_d[:, :])

            # ---------------- routing ---------------------------------------
            # logits = x @ gw computed as xh@gh + xl@gh + xh@gl (bf16 3-term,
            # fp32-accurate). gw-stationary: lhsT=[gh|gl] (16 cols), rhs = x
            # chunk (512 tokens) => logitsT [16, 512] in PSUM, then PE
            # transpose back to token-major and fold the two halves.
            lg = route.tile([128, MT, E], f32)
            msk = route.tile([128, MT], f32)
            ids32 = small.tile([128, MT], mybir.dt.int32)
            nc.gpsimd.iota(ids32[:], pattern=[[128, MT]], base=0, channel_multiplier=1)
            pos = small.tile([16, C // 16], mybir.dt.int32)
            nc.gpsimd.iota(pos[:], pattern=[[16, C // 16]], base=0, channel_multiplier=1)
            posf = small.tile([16, C // 16], f32)
            nc.vector.tensor_copy(out=posf[:], in_=pos[:])

            NXC = S // XCH    # x DMA chunks per tensor (4)
            NCH = S // RCH    # MM groups (8)
            MPC = RCH // 128  # token tiles per MM group (4)
            xh_tiles, xl_tiles = [], []
            xh_dmas, xl_dmas = [], []
            for q in range(NXC):
                xh = xr.tile([128, NTD, XCH], bf16, tag="xh")
                dh = nc.sync.dma_start(
                    out=xh[:],
                    in_=xhT_d[:, q * XCH : (q + 1) * XCH].rearrange("(dt p) s -> p dt s", p=128),
                )
                xl = xr.tile([128, NTD, XCH], bf16, tag="xl")
                dl = nc.scalar.dma_start(
                    out=xl[:],
                    in_=xlT_d[:, q * XCH : (q + 1) * XCH].rearrange("(dt p) s -> p dt s", p=128),
                )
                xh_tiles.append(xh)
                xl_tiles.append(xl)
                xh_dmas.append(dh)
                xl_dmas.append(dl)

            b1_sb = small.tile([128, NTF], f32)         # b1_sb[p,ft] = b1[ft*128+p]
            bd1 = nc.scalar.dma_start(out=b1_sb[:], in_=b1_d[:].rearrange("(ft p) -> p ft", p=128))
            add_dep_helper(bd1.ins, xl_dmas[-1].ins, sync=False, reason="xl stream first")
            b2_sb = small.tile([128, NTD], f32)
            nc.scalar.dma_start(out=b2_sb[:], in_=b2_d[:].rearrange("(dt p) -> p dt", p=128))

            for ch in range(NCH):
                q, half = divmod(ch, XCH // RCH)
                xh = xh_tiles[q][:, :, half * RCH : (half + 1) * RCH]
                xl = xl_tiles[q][:, :, half * RCH : (half + 1) * RCH]
                # two accumulation chains in separate PSUM banks:
                #   ps16 = sum_dt [gh|gl].T @ xh    (16 rows)
                #   ps8  = sum_dt  gh.T     @ xl    (8 rows)
                ps16 = psL.tile([16, RCH], f32, tag="lgps")
                for dt in range(NTD):
                    nc.tensor.matmul(
                        ps16[:],
                        lhsT=ghl_sb[:, dt, :],
                        rhs=xh[:, dt, :],
                        start=(dt == 0),
                        stop=(dt == NTD - 1),
                    )
                ps8 = psL.tile([E, RCH], f32, tag="lg8")
                for dt in range(NTD):
                    nc.tensor.matmul(
                        ps8[:],
                        lhsT=ghl_sb[:, dt, 0:E],
                        rhs=xl[:, dt, :],
                        start=(dt == 0),
                        stop=(dt == NTD - 1),
                    )
                lgT16 = route.tile([16, RCH], f32, tag="lgT16")
                nc.vector.tensor_copy(out=lgT16[:], in_=ps16[:])
                lgT8 = route.tile([E, RCH], f32, tag="lgT8")
                nc.vector.tensor_copy(out=lgT8[:], in_=ps8[:])
                for mm in range(MPC):
                    m = ch * MPC + mm
                    # fold hi/lo terms + transpose to token-major via two
                    # plain matmuls: psF = lgT16.T @ [I8;I8] + lgT8.T @ I8
                    psF = psT.tile([128, E], f32, tag="trps")
                    nc.tensor.matmul(psF[:], lhsT=lgT16[:, mm * 128 : (mm + 1) * 128],
                                     rhs=idn_sb[:], start=True, stop=False)
                    nc.tensor.matmul(psF[:], lhsT=lgT8[:, mm * 128 : (mm + 1) * 128],
                                     rhs=idn_sb[0:E, :], start=False, stop=True)
                    nc.vector.tensor_copy(out=lg[:, m, :], in_=psF[:])
                # per-chunk top-2 selection mask (gates applied host-side):
                # token is routed to the own expert (col 0) iff lg0 >= m2
                # (the second-highest logit).
                sl = slice(ch * MPC, (ch + 1) * MPC)
                lgc = lg[:, sl, :]
                m1c = route.tile([128, MPC], f32, tag="m1c")
                nc.vector.tensor_reduce(out=m1c[:], in_=lgc, axis=AX.X, op=OP.max)
                m1ap = m1c[:]
                m1b = bass.AP(tensor=m1ap.tensor, offset=m1ap.offset,
                              ap=[m1ap.ap[0], m1ap.ap[1], [0, E]])
                eqc = route.tile([128, MPC, E], f32, tag="eqc")
                nc.vector.tensor_tensor(out=eqc[:], in0=lgc, in1=m1b, op=OP.is_equal)
                nc.vector.tensor_scalar_mul(eqc[:], eqc[:], 1.0e30)
                nc.vector.tensor_sub(eqc[:], lgc, eqc[:])
                m2c = route.tile([128, MPC], f32, tag="m2c")
                nc.vector.tensor_reduce(out=m2c[:], in_=eqc[:], axis=AX.X, op=OP.max)
                nc.vector.tensor_tensor(out=msk[:, sl], in0=lgc[:, :, 0], in1=m2c[:],
                                        op=OP.is_ge)

            # ---------------- primary compaction (gates the FFN) ------------
            idsf = small.tile([128, MT], f32)
            nc.vector.tensor_copy(out=idsf[:], in_=ids32[:])
            nc.vector.tensor_scalar_add(idsf[:], idsf[:], 1.0)
            nc.vector.tensor_mul(idsf[:], idsf[:], msk[:])
            nc.vector.tensor_scalar_add(idsf[:], idsf[:], -1.0)
            ids16 = small.tile([128, MT], i16)
            nc.vector.tensor_copy(out=ids16[:], in_=idsf[:])
            ids_dram = dram.tile([128, MT], i16)
            d1 = nc.scalar.dma_start(out=ids_dram[:], in_=ids16[:])
            sg_in = small.tile([16, 8 * MT], i16)
            d2 = nc.scalar.dma_start(out=sg_in[:], in_=ids_dram[:].rearrange("(q v) m -> q (v m)", v=8))

            idxc0 = small.tile([16, C // 16], i16)
            nf = small.tile([1, 1], mybir.dt.uint32)
            sg1 = nc.gpsimd.sparse_gather(idxc0[:], sg_in[:], num_found=nf[:])
            # a second dummy gather right after sg1: triggers the gather
            # library reload CONCURRENT with the idx DMA hops below, so the
            # real gathers find it resident.
            jg2 = nc.gpsimd.dma_gather(
                out_ap=jxg[:], in_ap=xb_d[:, 0:128], idxs_ap=jgidx[:],
                num_idxs=128, num_idxs_reg=128, elem_size=128, elem_step=D,
                transpose=True, single_packet=False,
            )
            add_dep_helper(jg2.ins, sg1.ins, sync=False, reason="reload under hops")
            # gather indices: raw sparse_gather output clamped to [0, SENT].
            # Slots beyond num_found gather harmless in-bounds rows; the host
            # discards them via the cleaned idx output below, which is
            # computed OFF the critical path.
            idxg = small.tile([16, C // 16], i16)
            nc.vector.tensor_scalar(out=idxg[:], in0=idxc0[:], scalar1=0,
                                    scalar2=SENT, op0=OP.max, op1=OP.min)
            idxg_dram = dram.tile([16, C // 16], i16)
            gw1 = nc.scalar.dma_start(out=idxg_dram[:], in_=idxg[:])
            idx128 = small.tile([128, C // 16], i16)
            isrc = idxg_dram[:]
            irep = bass.AP(tensor=isrc.tensor, offset=isrc.offset,
                           ap=[[0, 8]] + list(isrc.ap))
            idx128_dma = nc.scalar.dma_start(out=idx128[:], in_=irep)
            # HW sparse_gather leaves garbage beyond num_found; mask to -1.
            # num_found scalar -> 16 partitions via a DMA round trip (a
            # partition_broadcast here would evict the gather library again).
            nf_dram = dram.tile([1, 1], mybir.dt.uint32)
            nfw = nc.scalar.dma_start(out=nf_dram[:], in_=nf[:])
            add_dep_helper(nfw.ins, idx128_dma.ins, sync=False,
                           reason="gather idx hops first")
            nf16u = small.tile([16, 1], mybir.dt.uint32)
            nsrc = nf_dram[:]
            nrep = bass.AP(tensor=nsrc.tensor, offset=nsrc.offset,
                           ap=[[0, 16], [1, 1]])
            nc.scalar.dma_start(out=nf16u[:], in_=nrep)
            nf16 = small.tile([16, 1], f32)
            nc.vector.tensor_copy(out=nf16[:], in_=nf16u[:])
            vld = small.tile([16, C // 16], f32)
            nc.vector.tensor_scalar(out=vld[:], in0=posf[:], scalar1=nf16[:, 0:1], scalar2=None,
                                    op0=OP.is_lt)
            vld16 = small.tile([16, C // 16], i16)
            nc.vector.tensor_copy(out=vld16[:], in_=vld[:])
            idxc = small.tile([16, C // 16], i16)
            nc.vector.tensor_scalar_add(idxc[:], idxc0[:], 1)
            nc.vector.tensor_mul(idxc[:], idxc[:], vld16[:])
            nc.vector.tensor_scalar_add(idxc[:], idxc[:], -1)
            # host-facing idx output: off the critical path
            nc.scalar.dma_start(out=idx_d[:, :], in_=idxc[:])

            do_gather = stage in ("gather", "full")
            do_ffn = stage == "full"
            # xg pieces: [128, NTD, GP] each, gathered separately so the FFN
            # can start on piece 0 while later pieces stream.
            xgs = []
            gis = []
            if do_gather:
                for g in range(NGP):
                    xg_g = wpool.tile([128, NTD, GP], bf16, tag=f"xg{g}")
                    gi = nc.gpsimd.dma_gather(
                        out_ap=xg_g[:], in_ap=xb_d[:, :],
                        idxs_ap=idx128[:, g * (GP // 16) : (g + 1) * (GP // 16)],
                        num_idxs=GP, num_idxs_reg=GP, elem_size=D, transpose=True,
                        single_packet=False,
                    )
                    if gis:
                        add_dep_helper(gi.ins, gis[-1].ins, sync=False,
                                       reason="gather piece order")
                    xgs.append(xg_g)
                    gis.append(gi)

            # ---------------- weights: bulk stream after x (sync queue) -----
            w1_sb = wpool.tile([128, NTD, F], bf16)     # w1_sb[p,dt,f] = w1t[dt*128+p, f]
            w2_sb = wpool.tile([128, NTF, D], bf16)     # w2_sb[p,ft,d] = w2t[ft*128+p, d]
            w1a = nc.sync.dma_start(out=w1_sb[:, :, 0 : F // 2],
                                    in_=w1_d[:, 0 : F // 2].rearrange("(dt p) f -> p dt f", p=128))
            w1b = nc.sync.dma_start(out=w1_sb[:, :, F // 2 : F],
                                    in_=w1_d[:, F // 2 : F].rearrange("(dt p) f -> p dt f", p=128))
            w2a = nc.sync.dma_start(out=w2_sb[:, :, 0 : D // 2],
                                    in_=w2_d[:, 0 : D // 2].rearrange("(ft p) d -> p ft d", p=128))
            w2b = nc.sync.dma_start(out=w2_sb[:, :, D // 2 : D],
                                    in_=w2_d[:, D // 2 : D].rearrange("(ft p) d -> p ft d", p=128))
            add_dep_helper(w1a.ins, xh_dmas[-1].ins, sync=True, reason="x stream first")
            add_dep_helper(w1b.ins, w1a.ins, sync=True, reason="weight order")
            add_dep_helper(w2a.ins, w1b.ins, sync=True, reason="weight order")
            add_dep_helper(w2b.ins, w2a.ins, sync=True, reason="weight order")

            if stage == "gather":
                for g in range(NGP):
                    nc.sync.dma_start(
                        out=yg_d[:, :, g * GP : (g + 1) * GP],
                        in_=xgs[g][:])
            # ---------------- FFN over token blocks -------------------------
            for blk in range(NBLK if do_ffn else 0):
                off = blk * TB
                g = off // GP
                goff = off - g * GP
                xg_blk = xgs[g]
                h_sb = small.tile([128, NTF, TB], bf16, tag="h")
                for ft in range(NTF):
                    hp = psH.tile([128, TB], f32, tag="hps")
                    for dt in range(NTD):
                        nc.tensor.matmul(
                            hp[:],
                            lhsT=w1_sb[:, dt, ft * 128 : (ft + 1) * 128],
                            rhs=xg_blk[:, dt, goff : goff + TB],
                            start=(dt == 0),
                            stop=(dt == NTD - 1),
                        )
                    nc.scalar.activation(out=h_sb[:, ft, :], in_=hp[:], func=ACT.Relu,
                                         bias=b1_sb[:, ft : ft + 1], scale=1.0)
                y_blk = ypool.tile([128, NTD, TB], bf16, tag="y")
                for dt in range(NTD):
                    yp = psY.tile([128, TB], f32, tag="yps")
                    for ft in range(NTF):
                        nc.tensor.matmul(
                            yp[:],
                            lhsT=w2_sb[:, ft, dt * 128 : (dt + 1) * 128],
                            rhs=h_sb[:, ft, :],
                            start=(ft == 0),
                            stop=(ft == NTF - 1),
                        )
                    nc.vector.tensor_scalar_add(y_blk[:, dt, :], yp[:], b2_sb[:, dt : dt + 1])
                nc.sync.dma_start(out=yg_d[:, :, off : off + TB], in_=y_blk[:])

    nc.compile()
    return nc


def _get_nc(C: int):
    import os
    stage = os.environ.get("KSTAGE", "full")
    key = (C, stage)
    if key not in _cache:
        _cache[key] = _build(C, stage)
    return _cache[key]


def kernel(x, gate_w, w1, b1, w2, b2, k):
    from concourse.bass_utils import run_bass_kernel_spmd

    assert int(k) == 2
    x = np.asarray(x, dtype=np.float32)
    gate_w = np.asarray(gate_w, dtype=np.float32)
    w1 = np.asarray(w1, dtype=np.float32)
    b1 = np.asarray(b1, dtype=np.float32)
    w2 = np.asarray(w2, dtype=np.float32)
    b2 = np.asarray(b2, dtype=np.float32)
    B, T, _ = x.shape
    xf = x.reshape(S, D)

    # capacity: per-expert token counts (host-side count only picks the
    # compiled capacity variant; routing itself happens on-device).
    # Gate weights are applied host-side during the merge (exact f32,
    # matching the reference's top-2 renormalized softmax).
    logits_host = xf @ gate_w.T
    top2 = np.argpartition(-logits_host, 2, axis=1)[:, :2]
    cnt = np.bincount(top2.reshape(-1), minlength=E).max()
    C = max(1152, int(math.ceil((cnt + 64) / 384.0)) * 384)
    topv = np.take_along_axis(logits_host, top2, axis=1)            # (S, 2)
    ex = np.exp(topv - topv.max(axis=1, keepdims=True))
    gsm = ex / ex.sum(axis=1, keepdims=True)
    gates = np.zeros((S, E), dtype=np.float32)
    np.put_along_axis(gates, top2, gsm.astype(np.float32), axis=1)  # (S, E)

    nc = _get_nc(C)

    xfT = np.ascontiguousarray(xf.T)
    xhT = xfT.astype(ml_dtypes.bfloat16)
    xlT = (xfT - xhT.astype(np.float32)).astype(ml_dtypes.bfloat16)
    xb = np.zeros((XROWS, D), dtype=ml_dtypes.bfloat16)
    xb[:S] = xhT.T
    idn = np.concatenate([np.eye(E, dtype=np.float32),
                          np.eye(E, dtype=np.float32)], axis=0)  # [16, 8]
    in_maps = []
    for c in range(E):
        perm = [c] + [e for e in range(E) if e != c]
        gwp = np.ascontiguousarray(gate_w[perm].T)      # [D, E] f32
        gh = gwp.astype(ml_dtypes.bfloat16)
        gl = (gwp - gh.astype(np.float32)).astype(ml_dtypes.bfloat16)
        ghl = np.concatenate([gh, gl], axis=1)          # [D, 16] bf16
        in_maps.append({
            "xhT": xhT,
            "xlT": xlT,
            "xb": xb,
            "ghl": np.ascontiguousarray(ghl),
            "idn": idn,
            "w1t": np.ascontiguousarray(w1[c].T).astype(ml_dtypes.bfloat16),
            "w2t": np.ascontiguousarray(w2[c].T).astype(ml_dtypes.bfloat16),
            "b1": b1[c].copy(),
            "b2": b2[c].copy(),
        })

    res = run_bass_kernel_spmd(nc, in_maps, core_ids=list(range(8)))
    global LAST_RES
    LAST_RES = res

    out = np.zeros((S, D), dtype=np.float32)
    for c in range(E):
        idx = np.asarray(res.results[c]["idx"])          # (16, C//16)
        order = idx.T.reshape(-1)                        # unwrap i = s*16+p
        valid = order >= 0
        tok = order[valid].astype(np.int64)
        yg = np.asarray(res.results[c]["yg"]).astype(np.float32)  # (128, NTD, C)
        yt = yg.transpose(1, 0, 2).reshape(D, -1)        # d = dt*128+p
        out[tok] += yt[:, valid].T * gates[tok, c][:, None]
    return out.reshape(B, T, D)



# revision 2
# speedup vs baseline: 1.3509x; 1.3509x over previous
"""MoE positionwise FFN (top-2 of 8 experts) on 8 TRN2 NeuronCores.

Strategy: expert-parallel, host-routed. The router (logits -> top-2 ->
softmax gates) is exact fp32 on host (as is the final scatter-add
combine, matching the reference semantics). Each core owns one expert:
the host gathers that expert's routed tokens into a compact [D, C]
bf16 input, and the device kernel is a pure dense FFN:

    h = relu(w1 @ x + b1)   (D -> F)
    y = w2 @ h + b2         (F -> D)

over C tokens in blocks of TB, weight-stationary bf16 matmuls, fp32
PSUM accumulation. Weights (16 MB/core) stream on the sync DMA queue
concurrently with the first block's compute; x and biases stream on
the scalar queue. The PE is pre-warmed so the HAM clock gate is at
8/8 when the real matmuls start.

Self-contained: hardcodes shapes for B=2,T=2048,D=1024,F=4096,E=8,K=2.
"""
import math

import numpy as np
import ml_dtypes

S = 4096
D = 1024
F = 4096
E = 8
NTD = D // 128   # 8 d-tiles
NTF = F // 128   # 32 f-tiles

_cache: dict = {}
LAST_RES = None


def _build(C: int, TB: int):
    import concourse.bacc as bacc
    import concourse.tile as tile
    import concourse.mybir as mybir
    from concourse.tile import add_dep_helper

    f32 = mybir.dt.float32
    bf16 = mybir.dt.bfloat16
    ACT = mybir.ActivationFunctionType

    assert C % TB == 0
    NBLK = C // TB

    nc = bacc.Bacc("TRN2", target_bir_lowering=False, debug=False, num_devices=8)

    xgT_d = nc.dram_tensor("xgT", [D, C], bf16, kind="ExternalInput")
    w1_d = nc.dram_tensor("w1t", [D, F], bf16, kind="ExternalInput")
    w2_d = nc.dram_tensor("w2t", [F, D], bf16, kind="ExternalInput")
    b1_d = nc.dram_tensor("b1", [F], f32, kind="ExternalInput")
    b2_d = nc.dram_tensor("b2", [D], f32, kind="ExternalInput")
    yg_d = nc.dram_tensor("yg", [128, NTD, C], bf16, kind="ExternalOutput")

    with tile.TileContext(nc) as tc:
        with (
            tc.tile_pool(name="wpool", bufs=1) as wpool,
            tc.tile_pool(name="xr", bufs=1) as xr,
            tc.tile_pool(name="small", bufs=1) as small,
            tc.tile_pool(name="hpool", bufs=2) as hpool,
            tc.tile_pool(name="ypool", bufs=2) as ypool,
            tc.tile_pool(name="psH", bufs=3, space="PSUM") as psH,
            tc.tile_pool(name="psY", bufs=2, space="PSUM") as psY,
            tc.tile_pool(name="psW", bufs=1, space="PSUM") as psW,
        ):
            # ---- PE pre-warm: trip the HAM activity window during the
            # initial DMA so real matmuls start at 2.4 GHz. Full-width
            # (M=128) matmuls; ~20 x 256 cols @ cold 1.2 GHz ~ 4.3 us.
            junk = small.tile([128, 256], bf16)
            nc.vector.memset(junk[:], 0.0)
            wps = psW.tile([128, 256], f32)
            for _ in range(20):
                nc.tensor.matmul(wps[:], lhsT=junk[:, 0:128], rhs=junk[:],
                                 start=True, stop=True)

            # ---- biases + x on the scalar queue ------------------------
            b1_sb = small.tile([128, NTF], f32)     # b1_sb[p,ft] = b1[ft*128+p]
            nc.scalar.dma_start(out=b1_sb[:], in_=b1_d[:].rearrange("(ft p) -> p ft", p=128))
            b2_sb = small.tile([128, NTD], f32)
            nc.scalar.dma_start(out=b2_sb[:], in_=b2_d[:].rearrange("(dt p) -> p dt", p=128))

            xg = xr.tile([128, NTD, C], bf16)
            xdmas = []
            for blk in range(NBLK):
                dx = nc.scalar.dma_start(
                    out=xg[:, :, blk * TB : (blk + 1) * TB],
                    in_=xgT_d[:, blk * TB : (blk + 1) * TB].rearrange(
                        "(dt p) s -> p dt s", p=128),
                )
                if xdmas:
                    add_dep_helper(dx.ins, xdmas[-1].ins, sync=False,
                                   reason="x block order")
                xdmas.append(dx)

            # ---- weights stream on the sync queue, in consumption order:
            # w1 chunks (block-0 h-phase chases them), then w2 chunks.
            w1_sb = wpool.tile([128, NTD, F], bf16)   # w1_sb[p,dt,f] = w1t[dt*128+p, f]
            w2_sb = wpool.tile([128, NTF, D], bf16)   # w2_sb[p,ft,d] = w2t[ft*128+p, d]
            wdmas = []
            FC = F // 4
            for i in range(4):
                dw = nc.sync.dma_start(
                    out=w1_sb[:, :, i * FC : (i + 1) * FC],
                    in_=w1_d[:, i * FC : (i + 1) * FC].rearrange(
                        "(dt p) f -> p dt f", p=128),
                )
                if wdmas:
                    add_dep_helper(dw.ins, wdmas[-1].ins, sync=True,
                                   reason="weight order")
                wdmas.append(dw)
            DC = D // 4
            for i in range(4):
                dw = nc.sync.dma_start(
                    out=w2_sb[:, :, i * DC : (i + 1) * DC],
                    in_=w2_d[:, i * DC : (i + 1) * DC].rearrange(
                        "(ft p) d -> p ft d", p=128),
                )
                add_dep_helper(dw.ins, wdmas[-1].ins, sync=True,
                               reason="weight order")
                wdmas.append(dw)

            # ---- FFN over token blocks ---------------------------------
            for blk in range(NBLK):
                off = blk * TB
                xg_blk = xg[:, :, off : off + TB]
                h_sb = hpool.tile([128, NTF, TB], bf16, tag="h")
                for ft in range(NTF):
                    hp = psH.tile([128, TB], f32, tag="hps")
                    for dt in range(NTD):
                        nc.tensor.matmul(
                            hp[:],
                            lhsT=w1_sb[:, dt, ft * 128 : (ft + 1) * 128],
                            rhs=xg_blk[:, dt, :],
                            start=(dt == 0),
                            stop=(dt == NTD - 1),
                        )
                    nc.scalar.activation(out=h_sb[:, ft, :], in_=hp[:], func=ACT.Relu,
                                         bias=b1_sb[:, ft : ft + 1], scale=1.0)
                y_blk = ypool.tile([128, NTD, TB], bf16, tag="y")
                for dt in range(NTD):
                    yp = psY.tile([128, TB], f32, tag="yps")
                    for ft in range(NTF):
                        nc.tensor.matmul(
                            yp[:],
                            lhsT=w2_sb[:, ft, dt * 128 : (dt + 1) * 128],
                            rhs=h_sb[:, ft, :],
                            start=(ft == 0),
                            stop=(ft == NTF - 1),
                        )
                    nc.vector.tensor_scalar_add(y_blk[:, dt, :], yp[:], b2_sb[:, dt : dt + 1])
                nc.sync.dma_start(out=yg_d[:, :, off : off + TB], in_=y_blk[:])

    nc.compile()
    return nc


def _get_nc(C: int, TB: int):
    key = (C, TB)
    if key not in _cache:
        _cache[key] = _build(C, TB)
    return _cache[key]


def kernel(x, gate_w, w1, b1, w2, b2, k):
    from concourse.bass_utils import run_bass_kernel_spmd

    assert int(k) == 2
    x = np.asarray(x, dtype=np.float32)
    gate_w = np.asarray(gate_w, dtype=np.float32)
    w1 = np.asarray(w1, dtype=np.float32)
    b1 = np.asarray(b1, dtype=np.float32)
    w2 = np.asarray(w2, dtype=np.float32)
    b2 = np.asarray(b2, dtype=np.float32)
    B, T, _ = x.shape
    xf = x.reshape(S, D)

    # Router (exact fp32, matching the reference's top-2 renormalized
    # softmax; gates applied host-side during the merge).
    logits = xf @ gate_w.T
    top2 = np.argpartition(-logits, 2, axis=1)[:, :2]
    topv = np.take_along_axis(logits, top2, axis=1)              # (S, 2)
    ex = np.exp(topv - topv.max(axis=1, keepdims=True))
    gsm = ex / ex.sum(axis=1, keepdims=True)
    gates = np.zeros((S, E), dtype=np.float32)
    np.put_along_axis(gates, top2, gsm.astype(np.float32), axis=1)

    sel = np.zeros((S, E), dtype=bool)
    np.put_along_axis(sel, top2, True, axis=1)
    toks = [np.nonzero(sel[:, e])[0] for e in range(E)]
    maxcnt = max(len(t) for t in toks)

    # capacity: 4 blocks, block size a multiple of 16
    TB = max(64, int(math.ceil(maxcnt / 4 / 16)) * 16)
    C = 4 * TB

    nc = _get_nc(C, TB)

    xfT16 = np.ascontiguousarray(xf.T).astype(ml_dtypes.bfloat16)  # [D, S]
    in_maps = []
    for c in range(E):
        tp = np.zeros(C, dtype=np.int64)
        tp[: len(toks[c])] = toks[c]
        in_maps.append({
            "xgT": np.ascontiguousarray(xfT16[:, tp]),
            "w1t": np.ascontiguousarray(w1[c].T).astype(ml_dtypes.bfloat16),
            "w2t": np.ascontiguousarray(w2[c].T).astype(ml_dtypes.bfloat16),
            "b1": b1[c].copy(),
            "b2": b2[c].copy(),
        })

    res = run_bass_kernel_spmd(nc, in_maps, core_ids=list(range(8)))
    global LAST_RES
    LAST_RES = res

    out = np.zeros((S, D), dtype=np.float32)
    for c in range(E):
        cnt = len(toks[c])
        yg = np.asarray(res.results[c]["yg"]).astype(np.float32)  # (128, NTD, C)
        yt = yg.transpose(1, 0, 2).reshape(D, C)                  # d = dt*128+p
        out[toks[c]] += yt[:, :cnt].T * gates[toks[c], c][:, None]
    return out.reshape(B, T, D)
